# revision 1
# baseline (speedup 1.0000x reference)
"""RGCN (2-layer, basis decomposition) link-predict encoder on 8 Trainium2 cores.

Strategy (dst-block sharding, v2 — batched dma_gather pipeline):
  - Host: W_r = sum_b w_comp[r,b] * bases[b] (tiny einsum); edges sharded by dst
    block (n_nodes/8 per core); per-core edge arrays laid out in a tile
    structure IDENTICAL across cores (counts padded to the max over cores) so
    one SPMD program serves all 8 cores. Self-loop handled in the epilogue.
  - Device, per layer:
      Phase 1 (transform): edges grouped by (src-half, etype). Batched
        dma_gather(transpose=True) pulls h[src]^T tiles straight from HBM in
        [d, e] layout (no on-chip transpose); per tile one matmul with W_etype
        (X^T as lhsT) -> M[e,d] in PSUM; ScalarE copies PSUM->SBUF bf16 with
        the per-edge norm as scale; messages written to DRAM in 16-tile blocks.
      Phase 2 (scatter): edges re-ordered as [msg-chunk][dst-window][tile]
        (chunks keep dma_gather's int16 indices in range). Batched dma_gather
        pulls message rows; VectorE builds a 0/1 indicator (iota==dst_local);
        matmul(lhsT=M[e,d], rhs=Ind[e,slot]) accumulates out^T[d,slot] per
        (window,chunk) in PSUM; partials are added into per-window SBUF
        accumulators. Self-loop: matmul(lhsT=loop_w, rhs=h_blk^T gather).
        Epilogue per window: ScalarE bias+ReLU (bias is per-partition in the
        transposed layout), PE transpose back, DMA out in 8-window blocks.
  - One AllGather of h1 (bf16) between layers; layer-2 output blocks are
    concatenated on the host.
"""

import os
import sys
import numpy as np

for _p in ("/opt/trn_rl_repo", "/root/.axon_site/_ro/trn_rl_repo"):
    if os.path.isdir(_p) and _p not in sys.path:
        sys.path.append(_p)

import ml_dtypes
import concourse.bass as bass
import concourse.mybir as mybir
import concourse.tile as tile
import concourse.bacc as bacc
from concourse.bass_utils import run_bass_kernel_spmd

P = 128
GB = 32          # gather batch: tiles per dma_gather call
MAX_CHUNK_TILES = 255   # keep int16 row indices < 32768


def _ceil_div(a, b):
    return (a + b - 1) // b


def _wrap_idx16(flat):
    """[T*128] int array -> [128, T*8] int16 in dma_gather's wrapped layout:
    index i lives at [i%16, i//16], replicated over the 8 16-partition stripes."""
    T = len(flat) // P
    a = np.asarray(flat, np.int16).reshape(T, 8, 16)        # [t, c, r]
    a = np.ascontiguousarray(a.transpose(2, 0, 1).reshape(16, T * 8))
    return np.ascontiguousarray(np.tile(a, (8, 1)))


def _batch_calls(n_tiles_per_region):
    """Split regions (list of (region_id, start_tile, n_tiles)) into gather
    calls of <= GB tiles that never cross a region boundary."""
    calls = []
    for rid, start, n in n_tiles_per_region:
        t = 0
        while t < n:
            k = min(GB, n - t)
            calls.append((rid, start + t, k))
            t += k
    return calls


def _preprocess(src, dst, etype, norm, n_nodes, n_rels, n_cores):
    NB = n_nodes // n_cores
    NW = _ceil_div(NB, P)
    half = _ceil_div(n_nodes, 2) if n_nodes > 32767 else n_nodes
    n_halves = 2 if n_nodes > 32767 else 1

    src = np.asarray(src, np.int64)
    dst = np.asarray(dst, np.int64)
    etype = np.asarray(etype, np.int64)
    norm = np.asarray(norm, np.float32).reshape(-1)

    # ---- phase-1 subgroups: (half, etype) ----
    NSG = n_halves * n_rels
    cores = []
    cnt_sg = np.zeros((n_cores, NSG), np.int64)
    for c in range(n_cores):
        m = (dst // NB) == c
        es, ed, ee, en = src[m], dst[m], etype[m], norm[m]
        sg = (es // half) * n_rels + ee
        o1 = np.argsort(sg, kind="stable")
        cnt_sg[c] = np.bincount(sg[o1], minlength=NSG)
        cores.append((es, ed, ee, en, o1, sg))

    T_sg = [int(_ceil_div(int(cnt_sg[:, g].max()), P)) for g in range(NSG)]
    base_sg = np.concatenate([[0], np.cumsum(T_sg)])
    T1 = int(base_sg[-1])
    rel_of_tile, half_of_tile = [], []
    for g in range(NSG):
        rel_of_tile += [g % n_rels] * T_sg[g]
        half_of_tile += [g // n_rels] * T_sg[g]

    # phase-1 gather calls: batches within each half
    h_regions = []
    for hf in range(n_halves):
        t0 = int(base_sg[hf * n_rels])
        t1 = int(base_sg[(hf + 1) * n_rels])
        if t1 > t0:
            h_regions.append((hf, t0, t1 - t0))
    p1_calls = _batch_calls(h_regions)

    # ---- msg chunks ----
    n_chunks = max(1, _ceil_div(T1, MAX_CHUNK_TILES))
    chunk_tiles = _ceil_div(T1, n_chunks)
    chunk_base = [min(k * chunk_tiles, T1) for k in range(n_chunks + 1)]

    # ---- phase-2 groups: [chunk][window] ----
    per_core_mid = []
    cnt_cw = np.zeros((n_cores, n_chunks, NW), np.int64)
    for c in range(n_cores):
        es, ed, ee, en, o1, sg = cores[c]
        ne = len(es)
        slot1 = np.empty(ne, np.int64)
        pos = 0
        for g in range(NSG):
            cnt = int(cnt_sg[c, g])
            slot1[o1[pos:pos + cnt]] = base_sg[g] * P + np.arange(cnt)
            pos += cnt
        chunk_of = np.searchsorted(np.asarray(chunk_base[1:]) * P, slot1, "right")
        w_of = (ed - c * NB) // P
        key = chunk_of * NW + w_of
        o2 = np.argsort(key, kind="stable")
        cnt_cw[c] = np.bincount(key[o2], minlength=n_chunks * NW).reshape(n_chunks, NW)
        per_core_mid.append((slot1, chunk_of, w_of, o2, key))

    T_cw = np.zeros((n_chunks, NW), np.int64)
    for k in range(n_chunks):
        for w in range(NW):
            T_cw[k, w] = _ceil_div(int(cnt_cw[:, k, w].max()), P)
    base_cw = np.concatenate([[0], np.cumsum(T_cw.reshape(-1))]).reshape(-1)
    T2 = int(base_cw[-1])

    p2_regions = []
    for k in range(n_chunks):
        t0 = int(base_cw[k * NW])
        t1 = int(base_cw[(k + 1) * NW]) if k + 1 < n_chunks else T2
        if t1 > t0:
            p2_regions.append((k, t0, t1 - t0))
    p2_calls = _batch_calls(p2_regions)

    # ---- per-core runtime tensors ----
    per_core = []
    for c in range(n_cores):
        es, ed, ee, en, o1, sg = cores[c]
        slot1, chunk_of, w_of, o2, key = per_core_mid[c]
        p1_idx = np.zeros(T1 * P, np.int64)
        p1_norm = np.zeros(T1 * P, np.float32)
        pos = 0
        for g in range(NSG):
            cnt = int(cnt_sg[c, g])
            eids = o1[pos:pos + cnt]
            slots = base_sg[g] * P + np.arange(cnt)
            p1_idx[slots] = es[eids] - (g // n_rels) * half
            p1_norm[slots] = en[eids]
            pos += cnt

        p2_idx = np.zeros(T2 * P, np.int64)
        p2_dst = np.full(T2 * P, -1.0, np.float32)
        pos = 0
        for k in range(n_chunks):
            for w in range(NW):
                cnt = int(cnt_cw[c, k, w])
                eids = o2[pos:pos + cnt]
                slots = base_cw[k * NW + w] * P + np.arange(cnt)
                p2_idx[slots] = slot1[eids] - chunk_base[k] * P
                p2_dst[slots] = (ed[eids] - c * NB) % P
                pos += cnt

        per_core.append(dict(
            p1i=_wrap_idx16(p1_idx),
            p1n=np.ascontiguousarray(p1_norm.reshape(T1, P).T),
            p2i=_wrap_idx16(p2_idx),
            p2d=np.ascontiguousarray(
                p2_dst.reshape(T2, P).T.astype(ml_dtypes.bfloat16)),
        ))

    selfi = _wrap_idx16(np.arange(NW * P) % NB)

    struct = dict(
        NB=NB, NW=NW, T1=T1, T2=T2, n_halves=n_halves, half=half,
        n_chunks=n_chunks, chunk_base=[int(x) for x in chunk_base],
        base_cw=[int(x) for x in base_cw],
        T_cw=[[int(T_cw[k, w]) for w in range(NW)] for k in range(n_chunks)],
        rel_of_tile=rel_of_tile, half_of_tile=half_of_tile,
        p1_calls=p1_calls, p2_calls=p2_calls,
        n_rels=n_rels, n_cores=n_cores,
    )
    return struct, per_core, selfi


def _build_program(struct, n_nodes, d, repeat=1):
    DBG = int(os.environ.get('KDBG_STAGE', '4'))
    TLSIM = bool(int(os.environ.get('KDBG_TLSIM', '0')))
    P2MODE = os.environ.get('KDBG_P2', 'full')
    NB, NW = struct["NB"], struct["NW"]
    T1, T2 = struct["T1"], struct["T2"]
    n_chunks, chunk_base = struct["n_chunks"], struct["chunk_base"]
    base_cw, T_cw = struct["base_cw"], struct["T_cw"]
    rel_of_tile = struct["rel_of_tile"]
    p1_calls, p2_calls = struct["p1_calls"], struct["p2_calls"]
    n_rels = struct["n_rels"]
    n_cores = struct["n_cores"]
    half, n_halves = struct["half"], struct["n_halves"]
    NGW = n_rels + 1
    f32, bf16, i16 = mybir.dt.float32, mybir.dt.bfloat16, mybir.dt.int16
    i32 = mybir.dt.int32
    Act = mybir.ActivationFunctionType

    nc = bacc.Bacc("TRN2", target_bir_lowering=False, debug=False,
                   num_devices=1 if TLSIM else n_cores)

    h0 = nc.dram_tensor("h0", [n_nodes, d], bf16, kind="ExternalInput")
    h0blk = nc.dram_tensor("h0blk", [NB, d], bf16, kind="ExternalInput")
    w1 = nc.dram_tensor("w1", [d, NGW * d], bf16, kind="ExternalInput")
    w2 = nc.dram_tensor("w2", [d, NGW * d], bf16, kind="ExternalInput")
    b1 = nc.dram_tensor("b1", [P, 1], f32, kind="ExternalInput")
    b2 = nc.dram_tensor("b2", [P, 1], f32, kind="ExternalInput")
    p1i = nc.dram_tensor("p1i", [P, T1 * 8], i16, kind="ExternalInput")
    p1n = nc.dram_tensor("p1n", [P, T1], f32, kind="ExternalInput")
    p2i = nc.dram_tensor("p2i", [P, T2 * 8], i16, kind="ExternalInput")
    p2d = nc.dram_tensor("p2d", [P, T2], bf16, kind="ExternalInput")
    sfi = nc.dram_tensor("sfi", [P, NW * 8], i16, kind="ExternalInput")
    out = nc.dram_tensor("out", [NB, d], f32, kind="ExternalOutput")

    msgs = [
        nc.dram_tensor(f"msg{k}", [(chunk_base[k + 1] - chunk_base[k]) * P, d],
                       bf16)
        for k in range(n_chunks)
    ]
    h1blk = nc.dram_tensor("h1blk", [NB, d], bf16)
    h1full = nc.dram_tensor("h1full", [n_cores * NB, d], bf16)

    with tile.TileContext(nc) as tc:
        with (
            tc.tile_pool(name="cst", bufs=1) as cst,
            tc.tile_pool(name="g1p", bufs=3) as g1p,
            tc.tile_pool(name="g2p", bufs=3) as g2p,
            tc.tile_pool(name="mbp", bufs=3) as mbp,
            tc.tile_pool(name="sfp", bufs=2) as sfp,
            tc.tile_pool(name="accp", bufs=NW) as accp,
            tc.tile_pool(name="indp", bufs=4) as indp,
            tc.tile_pool(name="obp", bufs=3) as obp,
            tc.tile_pool(name="mtp", bufs=4) as mtp,
            tc.tile_pool(name="wbp", bufs=2) as wbp,
            tc.tile_pool(name="ps_m", bufs=2, space="PSUM") as ps_m,
            tc.tile_pool(name="ps_o", bufs=2, space="PSUM") as ps_o,
            tc.tile_pool(name="ps_t", bufs=1, space="PSUM") as ps_t,
            tc.tile_pool(name="ps_x", bufs=3, space="PSUM") as ps_x,
        ):
            ident = cst.tile([P, P], bf16)
            ident32 = cst.tile([P, P], f32)
            for idt in (ident, ident32):
                nc.gpsimd.memset(idt[:], 0.0)
                nc.gpsimd.affine_select(
                    out=idt[:], in_=idt[:],
                    compare_op=mybir.AluOpType.not_equal, fill=1.0,
                    base=0, pattern=[[-1, P]], channel_multiplier=1,
                )
            iota32 = cst.tile([P, P], i32)
            nc.gpsimd.iota(iota32[:], pattern=[[1, P]], base=0, channel_multiplier=0)
            iota = cst.tile([P, P], bf16)
            nc.vector.tensor_copy(iota[:], iota32[:])
            p1i_sb = cst.tile([P, T1 * 8], i16)
            nc.sync.dma_start(p1i_sb[:], p1i[:, :])
            p1n_sb = cst.tile([P, T1], f32)
            nc.sync.dma_start(p1n_sb[:], p1n[:, :])
            p2i_sb = cst.tile([P, T2 * 8], i16)
            nc.sync.dma_start(p2i_sb[:], p2i[:, :])
            p2d_sb = cst.tile([P, T2], bf16)
            nc.sync.dma_start(p2d_sb[:], p2d[:, :])
            sfi_sb = cst.tile([P, NW * 8], i16)
            nc.sync.dma_start(sfi_sb[:], sfi[:, :])
            w1_sb = cst.tile([P, NGW * d], bf16)
            nc.sync.dma_start(w1_sb[:], w1[:, :])
            w2_sb = cst.tile([P, NGW * d], bf16)
            nc.sync.dma_start(w2_sb[:], w2[:, :])
            b1_sb = cst.tile([P, 1], f32)
            nc.sync.dma_start(b1_sb[:], b1[:, :])
            b2_sb = cst.tile([P, 1], f32)
            nc.sync.dma_start(b2_sb[:], b2[:, :])

            def layer(h_src, h_blk, w_sb, bias_sb, relu, out_dram, out_f32):
                stop_after = DBG
                skip_p1 = P2MODE == 'gatheronly'
                # ---- Phase 1: transform; messages to DRAM ----
                mb = None
                mb_t0 = 0
                for (hf, t0, ktiles) in ([] if skip_p1 else p1_calls):
                    ni = ktiles * P
                    xt = g1p.tile([P, GB * P], bf16, tag="g1")
                    nc.gpsimd.dma_gather(
                        out_ap=xt[:, :ni].rearrange("p (o ni) -> p o ni", o=1),
                        in_ap=h_src[hf * half : min((hf + 1) * half, n_nodes), :],
                        idxs_ap=p1i_sb[:, t0 * 8 : (t0 + ktiles) * 8],
                        num_idxs=ni, num_idxs_reg=ni,
                        elem_size=d, transpose=True, single_packet=False,
                    )
                    for k in range(ktiles):
                        t = t0 + k
                        r = rel_of_tile[t]
                        m_ps = ps_m.tile([P, d], f32, tag="mp", space="PSUM")
                        nc.tensor.matmul(
                            out=m_ps[:],
                            lhsT=xt[:, k * P : (k + 1) * P],
                            rhs=w_sb[:, r * d : (r + 1) * d],
                            start=True, stop=True,
                        )
                        if mb is None:
                            mb = mbp.tile([P, GB * d], bf16, tag="mb")
                            mb_t0 = t
                        nc.scalar.activation(
                            mb[:, (t - mb_t0) * d : (t - mb_t0 + 1) * d],
                            m_ps[:], Act.Copy, scale=p1n_sb[:, t : t + 1],
                        )
                        at_chunk_end = (t + 1) in chunk_base
                        if t - mb_t0 + 1 == GB or t == T1 - 1 or at_chunk_end:
                            nt = t - mb_t0 + 1
                            ck_w = 0
                            while chunk_base[ck_w + 1] <= mb_t0:
                                ck_w += 1
                            r0 = (mb_t0 - chunk_base[ck_w]) * P
                            nc.sync.dma_start(
                                msgs[ck_w][r0 : r0 + nt * P, :].rearrange(
                                    "(t p) d -> p t d", p=P),
                                mb[:, : nt * d].rearrange(
                                    "p (t d) -> p t d", d=d),
                            )
                            mb = None

                if stop_after <= 1:
                    dummy = obp.tile([P, P], f32 if out_f32 else bf16, tag="obt2")
                    nc.gpsimd.memset(dummy[:], 0.0)
                    for w in range(NW):
                        rows = min(P, NB - w * P)
                        nc.sync.dma_start(out_dram[w*P:w*P+rows, :], dummy[:rows, :])
                    return
                # ---- self-loop h_blk^T gather ----
                sxt = sfp.tile([P, NW * P], bf16, tag="sxt")
                for s0 in ([] if skip_p1 else range(0, NW, GB)):
                    kt = min(GB, NW - s0)
                    nc.gpsimd.dma_gather(
                        out_ap=sxt[:, s0 * P : (s0 + kt) * P].rearrange(
                            "p (o ni) -> p o ni", o=1),
                        in_ap=h_blk[:, :],
                        idxs_ap=sfi_sb[:, s0 * 8 : (s0 + kt) * 8],
                        num_idxs=kt * P, num_idxs_reg=kt * P,
                        elem_size=d, transpose=True, single_packet=False,
                    )

                # ---- Phase 2: scatter, chunk-major; per-window accumulators --
                accs = [None] * NW
                gather_of_tile = {}
                call_pos = [0]

                def ensure_gathered(tile_idx):
                    while (tile_idx not in gather_of_tile
                           and call_pos[0] < len(p2_calls)):
                        ck_, t0_, ktiles_ = p2_calls[call_pos[0]]
                        call_pos[0] += 1
                        ni_ = ktiles_ * P
                        g2 = g2p.tile([P, GB * d], bf16, tag="g2")
                        nc.gpsimd.dma_gather(
                            out_ap=g2[:, :ni_].rearrange(
                                "p (o ni) -> p o ni", o=1),
                            in_ap=msgs[ck_][:, :],
                            idxs_ap=p2i_sb[:, t0_ * 8 : (t0_ + ktiles_) * 8],
                            num_idxs=ni_, num_idxs_reg=ni_,
                            elem_size=d, transpose=True, single_packet=False,
                        )
                        for k in range(ktiles_):
                            gather_of_tile[t0_ + k] = (g2, k)

                for ck in range(n_chunks):
                    for w in range(NW):
                        ntl = T_cw[ck][w]
                        has_self = ck == 0
                        if ntl == 0 and not has_self:
                            continue
                        if P2MODE in ('gather', 'gatheronly'):
                            for k in range(ntl):
                                ensure_gathered(base_cw[ck * NW + w] + k)
                            continue
                        o_ps = ps_o.tile([P, P], f32, tag="op", space="PSUM")
                        first = True
                        for k in range(ntl):
                            t = base_cw[ck * NW + w] + k
                            ensure_gathered(t)
                            g2, kk = gather_of_tile[t]
                            ind = indp.tile([P, P], bf16, tag="ind")
                            nc.vector.tensor_tensor(
                                out=ind[:], in0=iota[:],
                                in1=p2d_sb[:, t : t + 1].to_broadcast([P, P]),
                                op=mybir.AluOpType.is_equal,
                            )
                            if P2MODE == 'ind':
                                continue
                            mt_ps = ps_x.tile([P, P], bf16, tag="tp2",
                                              space="PSUM")
                            nc.tensor.transpose(
                                out=mt_ps[:], in_=g2[:, kk * P : (kk + 1) * P],
                                identity=ident[:])
                            mt = mtp.tile([P, P], bf16, tag="mt")
                            nc.scalar.activation(mt[:], mt_ps[:], Act.Copy)
                            nc.tensor.matmul(
                                out=o_ps[:],
                                lhsT=mt[:],
                                rhs=ind[:],
                                start=first,
                                stop=(k == ntl - 1) and not has_self,
                            )
                            first = False
                        if P2MODE == 'ind':
                            continue
                        if has_self:
                            nc.tensor.matmul(
                                out=o_ps[:],
                                lhsT=w_sb[:, n_rels * d : (n_rels + 1) * d],
                                rhs=sxt[:, w * P : (w + 1) * P],
                                start=first, stop=True,
                            )
                        if accs[w] is None:
                            acc = accp.tile([P, P], f32, tag="acc")
                            nc.vector.tensor_copy(acc[:], o_ps[:])
                            accs[w] = acc
                        else:
                            nc.vector.tensor_add(accs[w][:], accs[w][:], o_ps[:])

                if stop_after <= 2:
                    dummy = obp.tile([P, P], f32 if out_f32 else bf16, tag="obt2")
                    nc.gpsimd.memset(dummy[:], 0.0)
                    for w in range(NW):
                        rows = min(P, NB - w * P)
                        nc.sync.dma_start(out_dram[w*P:w*P+rows, :], dummy[:rows, :])
                    return
                # ---- epilogue: bias (+relu), transpose back, write out ----
                ob = None
                ob_w0 = 0
                WB = 8
                odt = f32 if out_f32 else bf16
                for w in range(NW):
                    obT = obp.tile([P, P], odt, tag="obt")
                    if relu:
                        nc.scalar.activation(
                            obT[:], accs[w][:], Act.Relu, bias=bias_sb[:, 0:1])
                    else:
                        nc.vector.tensor_scalar_add(
                            obT[:], accs[w][:], bias_sb[:, 0:1])
                    t_ps = ps_t.tile([P, P], odt, tag="tp", space="PSUM")
                    nc.tensor.transpose(
                        out=t_ps[:], in_=obT[:],
                        identity=ident32[:] if out_f32 else ident[:])
                    if ob is None:
                        ob = wbp.tile(
                            [P, WB * d], f32 if out_f32 else bf16, tag="wb")
                        ob_w0 = w
                    nc.vector.tensor_copy(
                        ob[:, (w - ob_w0) * d : (w - ob_w0 + 1) * d], t_ps[:])
                    if w - ob_w0 + 1 == WB or w == NW - 1:
                        nw_ = w - ob_w0 + 1
                        rows = min(nw_ * P, NB - ob_w0 * P)
                        if rows % P == 0:
                            nc.sync.dma_start(
                                out_dram[ob_w0 * P : ob_w0 * P + rows, :]
                                .rearrange("(t p) d -> p t d", p=P),
                                ob[:, : nw_ * d].rearrange(
                                    "p (t d) -> p t d", d=d),
                            )
                        else:
                            full = (rows // P) * P
                            if full:
                                nc.sync.dma_start(
                                    out_dram[ob_w0 * P : ob_w0 * P + full, :]
                                    .rearrange("(t p) d -> p t d", p=P),
                                    ob[:, : full // P * d].rearrange(
                                        "p (t d) -> p t d", d=d),
                                )
                            rem = rows - full
                            nc.sync.dma_start(
                                out_dram[ob_w0 * P + full
                                         : ob_w0 * P + rows, :],
                                ob[:rem, full // P * d
                                   : (full // P + 1) * d],
                            )
                        ob = None

            for _rep in range(repeat):
                layer(h0, h0blk, w1_sb, b1_sb, True, h1blk, False)
                if DBG == 3:
                    dummy2 = obp.tile([P, P], f32, tag="obt3")
                    nc.gpsimd.memset(dummy2[:], 0.0)
                    for w in range(NW):
                        rows = min(P, NB - w * P)
                        nc.sync.dma_start(out[w*P:w*P+rows, :], dummy2[:rows, :])
                    continue
                if not TLSIM and not bool(int(os.environ.get('KDBG_NOCOLL', '0'))):
                    nc.gpsimd.collective_compute(
                        "AllGather", mybir.AluOpType.bypass,
                        replica_groups=[list(range(n_cores))],
                        ins=[h1blk.ap().opt()], outs=[h1full.ap().opt()],
                    )
                layer(h1full, h1blk, w2_sb, b2_sb, False, out, True)

    nc.finalize()
    return nc


_CACHE = {}


def _get_program(struct, n_nodes, d, repeat=1):
    key = (n_nodes, d, struct["T1"], struct["T2"], tuple(struct["rel_of_tile"]),
           tuple(np.asarray(struct["base_cw"]).ravel()), struct["n_cores"], repeat)
    if key not in _CACHE:
        _CACHE[key] = _build_program(struct, n_nodes, d, repeat)
    return _CACHE[key]


def prepare(h_ids, src, dst, etype, norm, embedding,
            w_comp1, bases1, loop_w1, bias1,
            w_comp2, bases2, loop_w2, bias2, n_cores=8):
    h_ids = np.asarray(h_ids).astype(np.int64)
    src = np.asarray(src).astype(np.int64)
    dst = np.asarray(dst).astype(np.int64)
    etype = np.asarray(etype).astype(np.int64)
    norm = np.asarray(norm, dtype=np.float32)
    embedding = np.asarray(embedding, dtype=np.float32)
    n_nodes, d = embedding.shape
    n_rels = np.asarray(w_comp1).shape[0]
    NB = n_nodes // n_cores

    W1 = np.einsum("rb,bio->rio", np.asarray(w_comp1, np.float64),
                   np.asarray(bases1, np.float64)).astype(np.float32)
    W2 = np.einsum("rb,bio->rio", np.asarray(w_comp2, np.float64),
                   np.asarray(bases2, np.float64)).astype(np.float32)
    W1 = np.concatenate([W1, np.asarray(loop_w1, np.float32)[None]], 0)
    W2 = np.concatenate([W2, np.asarray(loop_w2, np.float32)[None]], 0)
    w1_dev = np.ascontiguousarray(
        np.transpose(W1, (1, 0, 2)).reshape(d, (n_rels + 1) * d)
    ).astype(ml_dtypes.bfloat16)
    w2_dev = np.ascontiguousarray(
        np.transpose(W2, (1, 0, 2)).reshape(d, (n_rels + 1) * d)
    ).astype(ml_dtypes.bfloat16)
    b1_dev = np.asarray(bias1, np.float32).reshape(P, 1).copy()
    b2_dev = np.asarray(bias2, np.float32).reshape(P, 1).copy()
    h0 = embedding[h_ids].astype(ml_dtypes.bfloat16)

    struct, per_core, selfi = _preprocess(
        src, dst, etype, norm, n_nodes, n_rels, n_cores)

    in_maps = []
    for c in range(n_cores):
        pc = per_core[c]
        in_maps.append({
            "h0": h0, "h0blk": np.ascontiguousarray(h0[c * NB:(c + 1) * NB]),
            "w1": w1_dev, "w2": w2_dev, "b1": b1_dev, "b2": b2_dev,
            "p1i": pc["p1i"], "p1n": pc["p1n"],
            "p2i": pc["p2i"], "p2d": pc["p2d"], "sfi": selfi,
        })
    return struct, in_maps, n_nodes, d


def run(h_ids, src, dst, etype, norm, embedding,
        w_comp1, bases1, loop_w1, bias1,
        w_comp2, bases2, loop_w2, bias2,
        n_cores=8, trace=False):
    struct, in_maps, n_nodes, d = prepare(
        h_ids, src, dst, etype, norm, embedding,
        w_comp1, bases1, loop_w1, bias1,
        w_comp2, bases2, loop_w2, bias2, n_cores)
    nc = _get_program(struct, n_nodes, d)
    res = run_bass_kernel_spmd(
        nc, in_maps, core_ids=list(range(n_cores)), trace=trace)
    blocks = [res.results[c]["out"] for c in range(n_cores)]
    full = np.concatenate(blocks, 0)[:n_nodes]
    if trace:
        return full, res
    return full


def kernel(h_ids, src, dst, etype, norm, embedding,
           w_comp1, bases1, loop_w1, bias1,
           w_comp2, bases2, loop_w2, bias2):
    return run(h_ids, src, dst, etype, norm, embedding,
               w_comp1, bases1, loop_w1, bias1,
               w_comp2, bases2, loop_w2, bias2)

